# revision 2
# baseline (speedup 1.0000x reference)
"""Trainium2 Bass kernel v4 for nn_BaselineDNN (embedding pooling + MLP).

Reference computation (B=2048, L=200, V=50000, D=300, H=128, C=20):
    emb = emb_table[x]                       # [B, L, D] gather
    s   = sum(emb, axis=1); mx = max(emb, axis=1)
    rep = concat([s / len^2, mx], -1)        # [B, 600]
    h   = relu(rep @ W_new.T + b_new)        # [B, 128]
    out = h @ W3.T + b3                      # [B, 20]

Design (v1 baseline ~608us, v2 ~469us, v3 ~305us):
  - Data-parallel over batch across 8 cores (256 rows/core, 2 groups of 128).
  - Host-side per-core vocab dedup: each core touches ~32K unique rows of the
    50K vocab (P[>32768] ~ 1e-18 for uniform x). Rows are remapped to ranks
    that fit int16 and the core's private table is uploaded as fp16 with a
    768B row pitch (the 256B-multiple stride dma_gather requires).
  - Gather via the vectorized SWDGE dma_gather ucode, reading only the real
    600B of each row (_dma_gather_raw skips bass's elem%256 assert; the
    ucode takes arbitrary byte lengths). Descriptor generation runs on the
    Q7 pair owning the call's queue; calls round-robin 4 queues so multiple
    generations overlap. A dummy 128-index gather up front pays the ~6us
    ucode-library IRAM load during the constant-DMA phase.
  - Token t of group g lands on partition t%128 (= batch row g*128 + t%128),
    column t//128, by construction of the host-built index list.
  - Sum folds run IN PLACE on the gather tile (no scratch); max level-1
    lands in a small scratch tile. All DVE ops in the gather shadow are
    2-source tensor_tensor/scalar_tensor_tensor: 1-source DVE ops pick the
    dual-port 2x_2P mode which hardware-locks against concurrent GpSimd
    SBUF activity (~24us stalls each in v2). PSUM->SBUF copies go on the
    idle ACT engine. Each group's finalization is emitted right after its
    chunks so group 0's tail hides under group 1's gathers.
  - MLP: PE transposes of rep + 2 fp32 matmuls.
"""

import numpy as np

import concourse.ap_utils as ap_utils
import concourse.bacc as bacc
import concourse.bass as bass
import concourse.mybir as mybir
import concourse.tile as tile
from concourse.bass_utils import run_bass_kernel_spmd

F16 = mybir.dt.float16
F32 = mybir.dt.float32
I16 = mybir.dt.int16

B, L, V, D, H, C = 2048, 200, 50000, 300, 128, 20
NCORES = 8
BL = B // NCORES          # 256 rows per core
P = 128                   # partitions
G = BL // P               # 2 groups of 128 rows
KD = 5                    # d-chunks of 128 for the 600-dim rep (640 padded)
DPAD = KD * P             # 640
VCAP = 32768              # per-core unique-row capacity (int16 index limit)
EPAD = 384                # fp16 row pitch in elements (768B, multiple of 256B)
IPC = P // 16             # idx int16 columns per token column (8)


def chunk_sizes(chunk):
    return [chunk] * (L // chunk) + ([L % chunk] if L % chunk else [])


def _dma_gather_raw(gp, out_ap, in_ap, idxs_ap, num_idxs, elem_size, elem_step,
                    queue_num):
    """bass.BassGpSimd.dma_gather (HBM source, no transpose) minus the
    elem_size_bytes%256 assert; elem bytes may be any size, only the row
    stride must be a 256B multiple. Mirrors bass.py:dma_gather."""
    gp._assert_queue_num(queue_num)
    assert idxs_ap.dtype == mybir.dt.int16
    assert in_ap.dtype == out_ap.dtype
    assert in_ap.space == bass.MemorySpace.DRAM
    assert idxs_ap.space == bass.MemorySpace.SBUF
    assert out_ap.space == bass.MemorySpace.SBUF
    assert ap_utils.ap_is_contiguous(out_ap.ap[1:])
    assert ap_utils.ap_is_contiguous(idxs_ap.ap[1:])
    assert in_ap.ap[-1][1] == out_ap.ap[-1][1] == elem_size
    assert out_ap.ap[0][1] * out_ap.ap[1][1] == num_idxs
    assert in_ap.ap[0][0] == elem_step
    stride_bytes = elem_step * mybir.dt.size(in_ap.dtype)
    stride_bytes_256 = stride_bytes // 256
    assert stride_bytes_256 * 256 == stride_bytes and stride_bytes_256 < 256

    _in_ap = gp.lower_ap_dma(in_ap, for_custom_bir_dma=True)
    _idxs_ap = gp.lower_ap(idxs_ap)
    _out_ap = gp.lower_ap(out_ap)
    return gp.add_instruction(
        mybir.InstDMAGatherAnt(
            name=gp.bass.get_next_instruction_name(),
            ins=[*_in_ap, _idxs_ap, gp.lower_val_access(gp.to_reg(num_idxs))],
            outs=[_out_ap],
            transpose=False,
            num_idxs=num_idxs,
            elem_size=elem_size,
            stride_bytes_256=stride_bytes_256,
            gen_mode=0,
            single_packet=False,
            queue_num=queue_num,
            sbuf_tokens_per_rank=0,
            sbuf_free_dim_per_rank=0,
            sbuf_free_dim_pad_per_rank=0,
            sbuf_byte_offset=0,
        )
    )


def build_program(gather_bufs: int = 10, nq: int = 4, chunk: int = 20,
                  scratch: int = 16384, pe_chunks: int = -1):
    """pe_chunks: number of chunks per group whose SUM runs on the PE
    (identity-matmul accumulation into PSUM) instead of DVE fold trees,
    offloading the DVE which otherwise co-bottlenecks with descriptor
    generation for SBUF ports."""
    chunks = chunk_sizes(chunk)
    nch = len(chunks)
    nc = bacc.Bacc(
        "TRN2", target_bir_lowering=False, debug=False, num_swdge_queues=nq,
        dynamic_dma_scratch_size=scratch,
    )

    tab = nc.dram_tensor("tab", [VCAP, EPAD], F16, kind="ExternalInput").ap()
    idx = nc.dram_tensor("idx", [P, G * L * IPC], I16, kind="ExternalInput").ap()
    invl = nc.dram_tensor("invl", [P, G], F32, kind="ExternalInput").ap()
    wnewt = nc.dram_tensor("wnewt", [KD, P, H], F32, kind="ExternalInput").ap()
    w3t = nc.dram_tensor("w3t", [H, C], F32, kind="ExternalInput").ap()
    bnew = nc.dram_tensor("bnew", [H, 1], F32, kind="ExternalInput").ap()
    b3 = nc.dram_tensor("b3", [C, 1], F32, kind="ExternalInput").ap()
    iden = nc.dram_tensor("iden", [P, P], F32, kind="ExternalInput").ap()
    out = nc.dram_tensor("out", [C, BL], F32, kind="ExternalOutput").ap()

    add = mybir.AluOpType.add
    mult = mybir.AluOpType.mult
    mx_op = mybir.AluOpType.max

    with tile.TileContext(nc) as tc:
        with (
            tc.tile_pool(name="const", bufs=1) as const_pool,
            tc.tile_pool(name="gath", bufs=gather_bufs) as gather_pool,
            tc.tile_pool(name="fold", bufs=1) as fold_pool,
            tc.tile_pool(name="work", bufs=2) as work_pool,
            tc.tile_pool(name="psum", bufs=2, space="PSUM") as psum_pool,
        ):
            # dummy gather: loads the extended-inst ucode library (~6us IRAM
            # DMA) while the constant DMAs stream in
            idx_sb = const_pool.tile([P, G * L * IPC], I16)
            nc.sync.dma_start(out=idx_sb[:], in_=idx[:])
            dummy_o = const_pool.tile([P, 1, D], F16)
            # Tile locks each DMASW sem lane (8, round-robin per SWDGE inst)
            # to one queue, so every gather -- this dummy included -- must
            # follow the same qn rotation to keep lane i%8 <-> queue i%nq
            # consistent (requires nq | 8). The dummy reuses the first real
            # token column as indices and exists to pay the ~6us ucode
            # library IRAM load under the constant-DMA phase.
            _dma_gather_raw(nc.gpsimd, dummy_o[:], tab[:, 0:D], idx_sb[:, 0:8],
                            P, D, EPAD, queue_num=0)
            qn0 = 1
            invl_sb = const_pool.tile([P, G], F32)
            nc.sync.dma_start(out=invl_sb[:], in_=invl[:])
            iden_sb = const_pool.tile([P, P], F32)
            nc.sync.dma_start(out=iden_sb[:], in_=iden[:])
            wnewt_sb = const_pool.tile([P, KD, H], F32)
            nc.sync.dma_start(out=wnewt_sb[:], in_=wnewt[:].transpose([1, 0, 2]))
            w3t_sb = const_pool.tile([H, C], F32)
            nc.sync.dma_start(out=w3t_sb[:], in_=w3t[:])
            bnew_sb = const_pool.tile([H, 1], F32)
            nc.sync.dma_start(out=bnew_sb[:], in_=bnew[:])
            b3_sb = const_pool.tile([C, 1], F32)
            nc.sync.dma_start(out=b3_sb[:], in_=b3[:])

            # [d-part, k-chunk, batch(2 groups)] transposed rep for the MLP
            rep_t = const_pool.tile([P, KD, BL], F32)
            zeros = const_pool.tile([P, D], F32)
            nc.vector.memset(zeros[:], 0.0)
            # fp16 identity for the PE sum chains (fp16 rhs needs fp16 lhsT)
            iden16 = const_pool.tile([P, P], F16)
            nc.vector.tensor_copy(out=iden16[:], in_=iden_sb[:])
            # rep pads are zeroed up front so no 1-source DVE op runs while
            # gathers are active
            reps = []
            for g in range(G):
                rep = work_pool.tile([P, DPAD], F32, tag="rep")
                nc.vector.memset(rep[:, 2 * D : DPAD], 0.0)
                reps.append(rep)

            def fold_cols(t, n, op, out2d=None):
                # pairwise-fold columns [0, n) of t down to column 0 in
                # place; the final 2->1 fold goes to 2D view out2d if given.
                while n > 2:
                    h = n // 2
                    nc.vector.tensor_tensor(
                        out=t[:, 0:h, :], in0=t[:, 0:h, :],
                        in1=t[:, n - h : n, :], op=op,
                    )
                    n -= h
                dst = out2d if out2d is not None else t[:, 0, :]
                nc.vector.tensor_tensor(
                    out=dst, in0=t[:, 0, :], in1=t[:, 1, :], op=op
                )

            qn = qn0 % nq
            for g in range(G):
                sump = work_pool.tile([P, nch, D], F16, tag="sump")
                maxp = work_pool.tile([P, nch, D], F16, tag="maxp")
                c0 = 0
                for ci, csz in enumerate(chunks):
                    gt = gather_pool.tile([P, chunk, D], F16, tag="gt")
                    nidx = csz * P
                    _dma_gather_raw(
                        nc.gpsimd,
                        gt[:, 0:csz, :],
                        tab[:, 0:D],
                        idx_sb[:, (g * L + c0) * IPC : (g * L + c0 + csz) * IPC],
                        nidx,
                        D,
                        EPAD,
                        queue_num=qn,
                    )
                    qn = (qn + 1) % nq

                    # max level 1 into scratch (always DVE)
                    fm = fold_pool.tile([P, chunk // 2, D], F16, tag="fm")
                    k = csz // 2
                    nc.vector.tensor_tensor(
                        out=fm[:, 0:k, :], in0=gt[:, 0:k, :],
                        in1=gt[:, k : 2 * k, :], op=mx_op,
                    )
                    fold_cols(fm, k, mx_op, out2d=maxp[:, ci, :])

                    if (ci % 3 != 0) if pe_chunks < 0 else (1 <= ci <= pe_chunks):
                        # sum on PE: identity-matmul accumulation into PSUM
                        psum_s = psum_pool.tile([P, D], F32, tag="psum_s")
                        for j in range(csz):
                            nc.tensor.matmul(
                                out=psum_s[:],
                                lhsT=iden16[:],
                                rhs=gt[:, j, :],
                                start=(j == 0),
                                stop=(j == csz - 1),
                            )
                        nc.scalar.copy(out=sump[:, ci, :], in_=psum_s[:])
                    else:
                        # sum on DVE: fold in place on the gather tile
                        nc.vector.tensor_tensor(
                            out=gt[:, 0:k, :], in0=gt[:, 0:k, :],
                            in1=gt[:, k : 2 * k, :], op=add,
                        )
                        fold_cols(gt, k, add, out2d=sump[:, ci, :])
                    c0 += csz

                # group finalization (runs under the next group's gathers):
                # fold chunk partials; max lands straight in rep[:, D:2D]
                rep = reps[g]
                fold_cols(maxp, nch, mx_op, out2d=rep[:, D : 2 * D])
                fold_cols(sump, nch, add)
                # mean_bug = s / len^2 (2-source form: (s * invl) + 0)
                nc.vector.scalar_tensor_tensor(
                    out=rep[:, 0:D],
                    in0=sump[:, 0, :],
                    scalar=invl_sb[:, g : g + 1],
                    in1=zeros[:],
                    op0=mult,
                    op1=add,
                )
                # transpose rep -> rep_t[:, k, g*128:(g+1)*128]
                for k in range(KD):
                    pt = psum_pool.tile([P, P], F32, tag="pt")
                    nc.tensor.transpose(
                        out=pt[:],
                        in_=rep[:, k * P : (k + 1) * P],
                        identity=iden_sb[:],
                    )
                    nc.scalar.copy(
                        out=rep_t[:, k, g * P : (g + 1) * P], in_=pt[:]
                    )

            # h = relu(rep @ W_new.T + b_new): out[h, b]
            psum_h = psum_pool.tile([P, BL], F32, tag="psum_h", bufs=1)
            for k in range(KD):
                nc.tensor.matmul(
                    out=psum_h[:],
                    lhsT=wnewt_sb[:, k, :],
                    rhs=rep_t[:, k, :],
                    start=(k == 0),
                    stop=(k == KD - 1),
                )
            h_sb = work_pool.tile([P, BL], F32)
            nc.scalar.activation(
                h_sb[:],
                psum_h[:],
                mybir.ActivationFunctionType.Relu,
                bias=bnew_sb[:],
                scale=1.0,
            )
            # logits = h @ W3.T + b3: out[c, b]
            psum_l = psum_pool.tile([C, BL], F32, tag="psum_l", bufs=1)
            nc.tensor.matmul(
                out=psum_l[:], lhsT=w3t_sb[:], rhs=h_sb[:], start=True, stop=True
            )
            lo_sb = work_pool.tile([C, BL], F32)
            nc.vector.tensor_scalar_add(lo_sb[:], psum_l[:], b3_sb[:])
            nc.sync.dma_start(out=out[:], in_=lo_sb[:])

    nc.compile()
    return nc


def make_in_maps(x, lengths, emb_table, W_new, b_new, W3, b3):
    x_np = np.asarray(x).astype(np.int64)
    emb_np = np.asarray(emb_table, dtype=np.float32)
    len_f = np.asarray(lengths).astype(np.float32)
    inv_len2 = (1.0 / (len_f * len_f)).astype(np.float32)

    wnewt_pad = np.zeros((DPAD, H), dtype=np.float32)
    wnewt_pad[: 2 * D, :] = np.asarray(W_new, dtype=np.float32).T
    wnewt_np = np.ascontiguousarray(wnewt_pad.reshape(KD, P, H))
    w3t_np = np.ascontiguousarray(np.asarray(W3, dtype=np.float32).T)
    bnew_np = np.asarray(b_new, dtype=np.float32).reshape(H, 1)
    b3_np = np.asarray(b3, dtype=np.float32).reshape(C, 1)
    iden_np = np.eye(P, dtype=np.float32)

    in_maps = []
    for c in range(NCORES):
        xl = x_np[c * BL : (c + 1) * BL]            # [256, 200]
        uniq, inv = np.unique(xl, return_inverse=True)
        assert len(uniq) <= VCAP, f"core {c}: {len(uniq)} unique rows > {VCAP}"
        inv = inv.reshape(BL, L).astype(np.int16)   # token ranks

        tab_np = np.zeros((VCAP, EPAD), dtype=np.float16)
        tab_np[: len(uniq), :D] = emb_np[uniq]

        # idx list per group: element i (= col*128 + p) is the rank of
        # x[g*128 + p, col]; SWDGE lane j of step s reads element s*16 + j.
        idx_np = np.zeros((P, G, L * IPC), dtype=np.int16)
        for g in range(G):
            lst = inv[g * P : (g + 1) * P].T.reshape(-1)      # [L*128] c-major
            blk = lst.reshape(L * IPC, 16).T                  # [16, L*8]
            idx_np[:, g, :] = np.tile(blk, (IPC, 1))
        idx_np = np.ascontiguousarray(idx_np.reshape(P, G * L * IPC))

        il = inv_len2[c * BL : (c + 1) * BL]
        invl_np = np.ascontiguousarray(il.reshape(G, P).T)
        in_maps.append(
            {
                "tab": tab_np,
                "idx": idx_np,
                "invl": invl_np,
                "wnewt": wnewt_np,
                "w3t": w3t_np,
                "bnew": bnew_np,
                "b3": b3_np,
                "iden": iden_np,
            }
        )
    return in_maps


def run(inputs, trace=False, gather_bufs=10, tmpdir=None, nq=4, chunk=20,
        scratch=16384, pe_chunks=-1):
    nc = build_program(gather_bufs=gather_bufs, nq=nq, chunk=chunk,
                       scratch=scratch, pe_chunks=pe_chunks)
    in_maps = make_in_maps(**inputs)
    res = run_bass_kernel_spmd(
        nc, in_maps, core_ids=list(range(NCORES)), trace=trace, tmpdir=tmpdir
    )
    outs = [res.results[c]["out"].T for c in range(NCORES)]  # each [256, 20]
    full = np.concatenate(outs, axis=0).astype(np.float32)
    return full, res


def kernel(**inputs) -> np.ndarray:
    full, _ = run(inputs, trace=False)
    return full


# revision 3
# speedup vs baseline: 1.2451x; 1.2451x over previous
"""Trainium2 Bass kernel v4 for nn_BaselineDNN (embedding pooling + MLP).

Reference computation (B=2048, L=200, V=50000, D=300, H=128, C=20):
    emb = emb_table[x]                       # [B, L, D] gather
    s   = sum(emb, axis=1); mx = max(emb, axis=1)
    rep = concat([s / len^2, mx], -1)        # [B, 600]
    h   = relu(rep @ W_new.T + b_new)        # [B, 128]
    out = h @ W3.T + b3                      # [B, 20]

Design (v1 baseline ~608us, v2 ~469us, v3 ~305us):
  - Data-parallel over batch across 8 cores (256 rows/core, 2 groups of 128).
  - Host-side per-core vocab dedup: each core touches ~32K unique rows of the
    50K vocab (P[>32768] ~ 1e-18 for uniform x). Rows are remapped to ranks
    that fit int16 and the core's private table is uploaded as fp16 with a
    768B row pitch (the 256B-multiple stride dma_gather requires).
  - Gather via the vectorized SWDGE dma_gather ucode, reading only the real
    600B of each row (_dma_gather_raw skips bass's elem%256 assert; the
    ucode takes arbitrary byte lengths). Descriptor generation runs on the
    Q7 pair owning the call's queue; calls round-robin 4 queues so multiple
    generations overlap. A dummy 128-index gather up front pays the ~6us
    ucode-library IRAM load during the constant-DMA phase.
  - Token t of group g lands on partition t%128 (= batch row g*128 + t%128),
    column t//128, by construction of the host-built index list.
  - Sum folds run IN PLACE on the gather tile (no scratch); max level-1
    lands in a small scratch tile. All DVE ops in the gather shadow are
    2-source tensor_tensor/scalar_tensor_tensor: 1-source DVE ops pick the
    dual-port 2x_2P mode which hardware-locks against concurrent GpSimd
    SBUF activity (~24us stalls each in v2). PSUM->SBUF copies go on the
    idle ACT engine. Each group's finalization is emitted right after its
    chunks so group 0's tail hides under group 1's gathers.
  - MLP: PE transposes of rep + 2 fp32 matmuls.
"""

import numpy as np

import concourse.ap_utils as ap_utils
import concourse.bacc as bacc
import concourse.bass as bass
import concourse.mybir as mybir
import concourse.tile as tile
from concourse.bass_utils import run_bass_kernel_spmd

F16 = mybir.dt.float16
F32 = mybir.dt.float32
I16 = mybir.dt.int16

B, L, V, D, H, C = 2048, 200, 50000, 300, 128, 20
NCORES = 8
BL = B // NCORES          # 256 rows per core
P = 128                   # partitions
G = BL // P               # 2 groups of 128 rows
KD = 5                    # d-chunks of 128 for the 600-dim rep (640 padded)
DPAD = KD * P             # 640
VCAP = 32768              # per-core unique-row capacity (int16 index limit)
EPAD = 384                # fp16 row pitch in elements (768B, multiple of 256B)
IPC = P // 16             # idx int16 columns per token column (8)


def chunk_sizes(chunk):
    if chunk == 20:
        # tapered tail: the last chunks' fold/sum chains run after the final
        # gather byte lands, so smaller final chunks shrink the serial tail
        return [20] * 9 + [12, 8]
    return [chunk] * (L // chunk) + ([L % chunk] if L % chunk else [])


def _dma_gather_raw(gp, out_ap, in_ap, idxs_ap, num_idxs, elem_size, elem_step,
                    queue_num):
    """bass.BassGpSimd.dma_gather (HBM source, no transpose) minus the
    elem_size_bytes%256 assert; elem bytes may be any size, only the row
    stride must be a 256B multiple. Mirrors bass.py:dma_gather."""
    gp._assert_queue_num(queue_num)
    assert idxs_ap.dtype == mybir.dt.int16
    assert in_ap.dtype == out_ap.dtype
    assert in_ap.space == bass.MemorySpace.DRAM
    assert idxs_ap.space == bass.MemorySpace.SBUF
    assert out_ap.space == bass.MemorySpace.SBUF
    assert ap_utils.ap_is_contiguous(out_ap.ap[1:])
    assert ap_utils.ap_is_contiguous(idxs_ap.ap[1:])
    assert in_ap.ap[-1][1] == out_ap.ap[-1][1] == elem_size
    assert out_ap.ap[0][1] * out_ap.ap[1][1] == num_idxs
    assert in_ap.ap[0][0] == elem_step
    stride_bytes = elem_step * mybir.dt.size(in_ap.dtype)
    stride_bytes_256 = stride_bytes // 256
    assert stride_bytes_256 * 256 == stride_bytes and stride_bytes_256 < 256

    _in_ap = gp.lower_ap_dma(in_ap, for_custom_bir_dma=True)
    _idxs_ap = gp.lower_ap(idxs_ap)
    _out_ap = gp.lower_ap(out_ap)
    return gp.add_instruction(
        mybir.InstDMAGatherAnt(
            name=gp.bass.get_next_instruction_name(),
            ins=[*_in_ap, _idxs_ap, gp.lower_val_access(gp.to_reg(num_idxs))],
            outs=[_out_ap],
            transpose=False,
            num_idxs=num_idxs,
            elem_size=elem_size,
            stride_bytes_256=stride_bytes_256,
            gen_mode=0,
            single_packet=False,
            queue_num=queue_num,
            sbuf_tokens_per_rank=0,
            sbuf_free_dim_per_rank=0,
            sbuf_free_dim_pad_per_rank=0,
            sbuf_byte_offset=0,
        )
    )


def build_program(gather_bufs: int = 10, nq: int = 4, chunk: int = 20,
                  scratch: int = 16384, pe_chunks: int = -1):
    """pe_chunks: number of chunks per group whose SUM runs on the PE
    (identity-matmul accumulation into PSUM) instead of DVE fold trees,
    offloading the DVE which otherwise co-bottlenecks with descriptor
    generation for SBUF ports."""
    chunks = chunk_sizes(chunk)
    nch = len(chunks)
    nc = bacc.Bacc(
        "TRN2", target_bir_lowering=False, debug=False, num_swdge_queues=nq,
        dynamic_dma_scratch_size=scratch,
    )

    tab = nc.dram_tensor("tab", [VCAP, EPAD], F16, kind="ExternalInput").ap()
    idx = nc.dram_tensor("idx", [P, G * L * IPC], I16, kind="ExternalInput").ap()
    invl = nc.dram_tensor("invl", [P, G], F32, kind="ExternalInput").ap()
    wnewt = nc.dram_tensor("wnewt", [KD, P, H], F32, kind="ExternalInput").ap()
    w3t = nc.dram_tensor("w3t", [H, C], F32, kind="ExternalInput").ap()
    bnew = nc.dram_tensor("bnew", [H, 1], F32, kind="ExternalInput").ap()
    b3 = nc.dram_tensor("b3", [C, 1], F32, kind="ExternalInput").ap()
    iden = nc.dram_tensor("iden", [P, P], F32, kind="ExternalInput").ap()
    out = nc.dram_tensor("out", [C, BL], F32, kind="ExternalOutput").ap()

    add = mybir.AluOpType.add
    mult = mybir.AluOpType.mult
    mx_op = mybir.AluOpType.max

    with tile.TileContext(nc) as tc:
        with (
            tc.tile_pool(name="const", bufs=1) as const_pool,
            tc.tile_pool(name="gath", bufs=gather_bufs) as gather_pool,
            tc.tile_pool(name="fold", bufs=1) as fold_pool,
            tc.tile_pool(name="work", bufs=2) as work_pool,
            tc.tile_pool(name="psum", bufs=2, space="PSUM") as psum_pool,
        ):
            # dummy gather: loads the extended-inst ucode library (~6us IRAM
            # DMA) while the constant DMAs stream in
            idx_sb = const_pool.tile([P, G * L * IPC], I16)
            nc.sync.dma_start(out=idx_sb[:], in_=idx[:])
            dummy_o = const_pool.tile([P, 1, D], F16)
            # Tile locks each DMASW sem lane (8, round-robin per SWDGE inst)
            # to one queue, so every gather -- this dummy included -- must
            # follow the same qn rotation to keep lane i%8 <-> queue i%nq
            # consistent (requires nq | 8). The dummy reuses the first real
            # token column as indices and exists to pay the ~6us ucode
            # library IRAM load under the constant-DMA phase.
            _dma_gather_raw(nc.gpsimd, dummy_o[:], tab[:, 0:D], idx_sb[:, 0:8],
                            P, D, EPAD, queue_num=0)
            qn0 = 1
            invl_sb = const_pool.tile([P, G], F32)
            nc.sync.dma_start(out=invl_sb[:], in_=invl[:])
            iden_sb = const_pool.tile([P, P], F32)
            nc.sync.dma_start(out=iden_sb[:], in_=iden[:])
            wnewt_sb = const_pool.tile([P, KD, H], F32)
            nc.sync.dma_start(out=wnewt_sb[:], in_=wnewt[:].transpose([1, 0, 2]))
            w3t_sb = const_pool.tile([H, C], F32)
            nc.sync.dma_start(out=w3t_sb[:], in_=w3t[:])
            bnew_sb = const_pool.tile([H, 1], F32)
            nc.sync.dma_start(out=bnew_sb[:], in_=bnew[:])
            b3_sb = const_pool.tile([C, 1], F32)
            nc.sync.dma_start(out=b3_sb[:], in_=b3[:])

            # [d-part, k-chunk, batch(2 groups)] transposed rep for the MLP
            rep_t = const_pool.tile([P, KD, BL], F32)
            zeros = const_pool.tile([P, D], F32)
            nc.vector.memset(zeros[:], 0.0)
            # fp16 identity for the PE sum chains (fp16 rhs needs fp16 lhsT)
            iden16 = const_pool.tile([P, P], F16)
            nc.vector.tensor_copy(out=iden16[:], in_=iden_sb[:])
            # rep pads are zeroed up front so no 1-source DVE op runs while
            # gathers are active
            reps = []
            for g in range(G):
                rep = work_pool.tile([P, DPAD], F32, tag="rep")
                nc.vector.memset(rep[:, 2 * D : DPAD], 0.0)
                reps.append(rep)

            def fold_cols(t, n, op, out2d=None):
                # pairwise-fold columns [0, n) of t down to column 0 in
                # place; the final 2->1 fold goes to 2D view out2d if given.
                while n > 2:
                    h = n // 2
                    nc.vector.tensor_tensor(
                        out=t[:, 0:h, :], in0=t[:, 0:h, :],
                        in1=t[:, n - h : n, :], op=op,
                    )
                    n -= h
                dst = out2d if out2d is not None else t[:, 0, :]
                nc.vector.tensor_tensor(
                    out=dst, in0=t[:, 0, :], in1=t[:, 1, :], op=op
                )

            qn = qn0 % nq
            for g in range(G):
                sump = work_pool.tile([P, nch, D], F16, tag="sump")
                maxp = work_pool.tile([P, nch, D], F16, tag="maxp")
                c0 = 0
                for ci, csz in enumerate(chunks):
                    gt = gather_pool.tile([P, chunk, D], F16, tag="gt")
                    nidx = csz * P
                    _dma_gather_raw(
                        nc.gpsimd,
                        gt[:, 0:csz, :],
                        tab[:, 0:D],
                        idx_sb[:, (g * L + c0) * IPC : (g * L + c0 + csz) * IPC],
                        nidx,
                        D,
                        EPAD,
                        queue_num=qn,
                    )
                    qn = (qn + 1) % nq

                    # max level 1 into scratch (always DVE)
                    fm = fold_pool.tile([P, chunk // 2, D], F16, tag="fm")
                    k = csz // 2
                    nc.vector.tensor_tensor(
                        out=fm[:, 0:k, :], in0=gt[:, 0:k, :],
                        in1=gt[:, k : 2 * k, :], op=mx_op,
                    )
                    fold_cols(fm, k, mx_op, out2d=maxp[:, ci, :])

                    if (ci % 3 != 0) if pe_chunks < 0 else (1 <= ci <= pe_chunks):
                        # sum on PE: identity-matmul accumulation into PSUM
                        psum_s = psum_pool.tile([P, D], F32, tag="psum_s")
                        for j in range(csz):
                            nc.tensor.matmul(
                                out=psum_s[:],
                                lhsT=iden16[:],
                                rhs=gt[:, j, :],
                                start=(j == 0),
                                stop=(j == csz - 1),
                            )
                        nc.scalar.copy(out=sump[:, ci, :], in_=psum_s[:])
                    else:
                        # sum on DVE: fold in place on the gather tile
                        nc.vector.tensor_tensor(
                            out=gt[:, 0:k, :], in0=gt[:, 0:k, :],
                            in1=gt[:, k : 2 * k, :], op=add,
                        )
                        fold_cols(gt, k, add, out2d=sump[:, ci, :])
                    c0 += csz

                # group finalization (runs under the next group's gathers):
                # fold chunk partials; max lands straight in rep[:, D:2D]
                rep = reps[g]
                fold_cols(maxp, nch, mx_op, out2d=rep[:, D : 2 * D])
                fold_cols(sump, nch, add)
                # mean_bug = s / len^2 (2-source form: (s * invl) + 0)
                nc.vector.scalar_tensor_tensor(
                    out=rep[:, 0:D],
                    in0=sump[:, 0, :],
                    scalar=invl_sb[:, g : g + 1],
                    in1=zeros[:],
                    op0=mult,
                    op1=add,
                )
                # transpose rep -> rep_t[:, k, g*128:(g+1)*128]
                for k in range(KD):
                    pt = psum_pool.tile([P, P], F32, tag="pt")
                    nc.tensor.transpose(
                        out=pt[:],
                        in_=rep[:, k * P : (k + 1) * P],
                        identity=iden_sb[:],
                    )
                    nc.scalar.copy(
                        out=rep_t[:, k, g * P : (g + 1) * P], in_=pt[:]
                    )

            # h = relu(rep @ W_new.T + b_new): out[h, b]
            psum_h = psum_pool.tile([P, BL], F32, tag="psum_h", bufs=1)
            for k in range(KD):
                nc.tensor.matmul(
                    out=psum_h[:],
                    lhsT=wnewt_sb[:, k, :],
                    rhs=rep_t[:, k, :],
                    start=(k == 0),
                    stop=(k == KD - 1),
                )
            h_sb = work_pool.tile([P, BL], F32)
            nc.scalar.activation(
                h_sb[:],
                psum_h[:],
                mybir.ActivationFunctionType.Relu,
                bias=bnew_sb[:],
                scale=1.0,
            )
            # logits = h @ W3.T + b3: out[c, b]
            psum_l = psum_pool.tile([C, BL], F32, tag="psum_l", bufs=1)
            nc.tensor.matmul(
                out=psum_l[:], lhsT=w3t_sb[:], rhs=h_sb[:], start=True, stop=True
            )
            lo_sb = work_pool.tile([C, BL], F32)
            nc.vector.tensor_scalar_add(lo_sb[:], psum_l[:], b3_sb[:])
            nc.sync.dma_start(out=out[:], in_=lo_sb[:])

    nc.compile()
    return nc


def make_in_maps(x, lengths, emb_table, W_new, b_new, W3, b3):
    x_np = np.asarray(x).astype(np.int64)
    emb_np = np.asarray(emb_table, dtype=np.float32)
    len_f = np.asarray(lengths).astype(np.float32)
    inv_len2 = (1.0 / (len_f * len_f)).astype(np.float32)

    wnewt_pad = np.zeros((DPAD, H), dtype=np.float32)
    wnewt_pad[: 2 * D, :] = np.asarray(W_new, dtype=np.float32).T
    wnewt_np = np.ascontiguousarray(wnewt_pad.reshape(KD, P, H))
    w3t_np = np.ascontiguousarray(np.asarray(W3, dtype=np.float32).T)
    bnew_np = np.asarray(b_new, dtype=np.float32).reshape(H, 1)
    b3_np = np.asarray(b3, dtype=np.float32).reshape(C, 1)
    iden_np = np.eye(P, dtype=np.float32)

    in_maps = []
    for c in range(NCORES):
        xl = x_np[c * BL : (c + 1) * BL]            # [256, 200]
        uniq, inv = np.unique(xl, return_inverse=True)
        assert len(uniq) <= VCAP, f"core {c}: {len(uniq)} unique rows > {VCAP}"
        inv = inv.reshape(BL, L).astype(np.int16)   # token ranks

        tab_np = np.zeros((VCAP, EPAD), dtype=np.float16)
        tab_np[: len(uniq), :D] = emb_np[uniq]

        # idx list per group: element i (= col*128 + p) is the rank of
        # x[g*128 + p, col]; SWDGE lane j of step s reads element s*16 + j.
        idx_np = np.zeros((P, G, L * IPC), dtype=np.int16)
        for g in range(G):
            lst = inv[g * P : (g + 1) * P].T.reshape(-1)      # [L*128] c-major
            blk = lst.reshape(L * IPC, 16).T                  # [16, L*8]
            idx_np[:, g, :] = np.tile(blk, (IPC, 1))
        idx_np = np.ascontiguousarray(idx_np.reshape(P, G * L * IPC))

        il = inv_len2[c * BL : (c + 1) * BL]
        invl_np = np.ascontiguousarray(il.reshape(G, P).T)
        in_maps.append(
            {
                "tab": tab_np,
                "idx": idx_np,
                "invl": invl_np,
                "wnewt": wnewt_np,
                "w3t": w3t_np,
                "bnew": bnew_np,
                "b3": b3_np,
                "iden": iden_np,
            }
        )
    return in_maps


def run(inputs, trace=False, gather_bufs=10, tmpdir=None, nq=4, chunk=20,
        scratch=16384, pe_chunks=-1):
    nc = build_program(gather_bufs=gather_bufs, nq=nq, chunk=chunk,
                       scratch=scratch, pe_chunks=pe_chunks)
    in_maps = make_in_maps(**inputs)
    res = run_bass_kernel_spmd(
        nc, in_maps, core_ids=list(range(NCORES)), trace=trace, tmpdir=tmpdir
    )
    outs = [res.results[c]["out"].T for c in range(NCORES)]  # each [256, 20]
    full = np.concatenate(outs, axis=0).astype(np.float32)
    return full, res


def kernel(**inputs) -> np.ndarray:
    full, _ = run(inputs, trace=False)
    return full
